# revision 4
# baseline (speedup 1.0000x reference)
"""Fused causal MHA (B=2,T=2048,D=1024,H=16) on 8 TRN2 NeuronCores.

Tensor-parallel over heads: each core owns 2 heads (128 qkv dims).
Per core: qkv projection (transposed layouts), RoPE, causal flash-style
attention without max-subtraction (scores ~ N(0,1), exp is safe), output
projection against the row-shard of Wout. Host sums the 8 partial outputs.

All matmuls run in bf16 (inputs quantized on host / on-chip), f32 PSUM
accumulation. Softmax exp in f32 on ACT directly from PSUM.
"""
import sys

sys.path.insert(0, "/opt/trn_rl_repo")

import numpy as np
import ml_dtypes

B, T, D, H = 2, 2048, 1024, 16
HD = 64
NCORES = 8
HLOC = H // NCORES          # heads per core = 2
DH = HLOC * HD              # local qkv dims per tensor = 128
BT = B * T                  # 4096 tokens
TI = 512                    # query tile (free dim)
TJ = 128                    # key tile (partitions)
BF16 = ml_dtypes.bfloat16

_built = {}


def _build():
    import concourse.bass as bass
    import concourse.bacc as bacc
    import concourse.tile as tile
    import concourse.mybir as mybir
    from contextlib import ExitStack

    f32 = mybir.dt.float32
    bf16 = mybir.dt.bfloat16
    Exp = mybir.ActivationFunctionType.Exp

    nc = bacc.Bacc("TRN2", target_bir_lowering=False, debug=False,
                   num_devices=NCORES)

    xT_d = nc.dram_tensor("xT", [D, BT], bf16, kind="ExternalInput")
    wqk_d = nc.dram_tensor("wqkT", [D, 2 * DH], bf16, kind="ExternalInput")
    wv_d = nc.dram_tensor("wvT", [D, DH], bf16, kind="ExternalInput")
    wout_d = nc.dram_tensor("woutT", [DH, D], bf16, kind="ExternalInput")
    cos_d = nc.dram_tensor("cosb", [DH, BT], f32, kind="ExternalInput")
    sin_d = nc.dram_tensor("sinb", [DH, BT], f32, kind="ExternalInput")
    mask_d = nc.dram_tensor("maskt", [TJ, 4 * TI], bf16, kind="ExternalInput")
    y_d = nc.dram_tensor("y", [BT, D], f32, kind="ExternalOutput")

    with tile.TileContext(nc) as tc, ExitStack() as ctx:
        px = ctx.enter_context(tc.tile_pool(name="px", bufs=8))
        pwq = ctx.enter_context(tc.tile_pool(name="pwq", bufs=8))
        pwv = ctx.enter_context(tc.tile_pool(name="pwv", bufs=8))
        pmisc = ctx.enter_context(tc.tile_pool(name="pmisc", bufs=1))
        pqk = ctx.enter_context(tc.tile_pool(name="pqk", bufs=16))
        pv = ctx.enter_context(tc.tile_pool(name="pv", bufs=32))
        pat = ctx.enter_context(tc.tile_pool(name="pat", bufs=8))
        ptmp = ctx.enter_context(tc.tile_pool(name="ptmp", bufs=3))
        pp = ctx.enter_context(tc.tile_pool(name="pp", bufs=4))
        pr = ctx.enter_context(tc.tile_pool(name="pr", bufs=4))
        psA = ctx.enter_context(
            tc.tile_pool(name="psA", bufs=2, space=bass.MemorySpace.PSUM))
        psS = ctx.enter_context(
            tc.tile_pool(name="psS", bufs=3, space=bass.MemorySpace.PSUM))
        psV = ctx.enter_context(
            tc.tile_pool(name="psV", bufs=3, space=bass.MemorySpace.PSUM))

        # ---- persistent loads ----
        sb_x = []
        for dt in range(8):
            t = px.tile([128, BT], bf16, tag="x")
            nc.sync.dma_start(t[:], xT_d.ap()[dt * 128:(dt + 1) * 128, :])
            sb_x.append(t)
        sb_wqk = []
        for dt in range(8):
            t = pwq.tile([128, 2 * DH], bf16, tag="wqk")
            nc.sync.dma_start(t[:], wqk_d.ap()[dt * 128:(dt + 1) * 128, :])
            sb_wqk.append(t)
        sb_wv = []
        for dt in range(8):
            t = pwv.tile([128, DH], bf16, tag="wv")
            nc.sync.dma_start(t[:], wv_d.ap()[dt * 128:(dt + 1) * 128, :])
            sb_wv.append(t)
        sb_wout = pmisc.tile([DH, D], bf16, tag="wout")
        nc.sync.dma_start(sb_wout[:], wout_d.ap())
        sb_cos = pmisc.tile([DH, BT], f32, tag="cos")
        nc.sync.dma_start(sb_cos[:], cos_d.ap())
        sb_sin = pmisc.tile([DH, BT], f32, tag="sin")
        nc.sync.dma_start(sb_sin[:], sin_d.ap())
        sb_mask = pmisc.tile([TJ, 4 * TI], bf16, tag="mask")
        nc.sync.dma_start(sb_mask[:], mask_d.ap())

        # ---- qkv projection ----
        # q/k in transposed layout [dims(2 heads x 64), tokens], with RoPE.
        sb_qk = {}
        for e in range(2):      # 0 = q, 1 = k
            for tt in range(BT // TI):
                ps = psA.tile([128, TI], f32, tag="ps")
                for dt in range(8):
                    nc.tensor.matmul(
                        ps[:],
                        sb_wqk[dt][:, e * DH:(e + 1) * DH],
                        sb_x[dt][:, tt * TI:(tt + 1) * TI],
                        start=(dt == 0), stop=(dt == 7))
                # DMA cannot read PSUM: stage to SBUF, then do the
                # rotate_half partition shift via SBUF->SBUF DMAs.
                qf = ptmp.tile([128, TI], f32, tag="qf")
                nc.vector.tensor_copy(qf[:], ps[:])
                qs = ptmp.tile([128, TI], f32, tag="qs")
                nc.sync.dma_start(qs[0:32, :], qf[32:64, :])
                nc.sync.dma_start(qs[32:64, :], qf[0:32, :])
                nc.sync.dma_start(qs[64:96, :], qf[96:128, :])
                nc.sync.dma_start(qs[96:128, :], qf[64:96, :])
                m1 = ptmp.tile([128, TI], f32, tag="m1")
                nc.vector.tensor_mul(m1[:], qf[:],
                                     sb_cos[:, tt * TI:(tt + 1) * TI])
                m2 = ptmp.tile([128, TI], f32, tag="m2")
                nc.vector.tensor_mul(m2[:], qs[:],
                                     sb_sin[:, tt * TI:(tt + 1) * TI])
                qk = pqk.tile([128, TI], bf16, tag="qk")
                nc.vector.tensor_add(qk[:], m1[:], m2[:])
                sb_qk[(e, tt)] = qk

        # v in natural layout [tokens, dims] + a ones column for the
        # softmax denominator (AV matmul row 64 sums the attn weights).
        sb_v = []
        for tt in range(BT // TJ):
            psv = psA.tile([128, DH], f32, tag="ps")
            for dt in range(8):
                nc.tensor.matmul(
                    psv[:],
                    sb_x[dt][:, tt * TJ:(tt + 1) * TJ],
                    sb_wv[dt][:],
                    start=(dt == 0), stop=(dt == 7))
            vt = pv.tile([128, 2, HD + 1], bf16, tag="v")
            nc.vector.tensor_copy(vt[:, :, 0:HD],
                                  psv[:].rearrange("p (h d) -> p h d", h=2))
            nc.vector.memset(vt[:, :, HD:HD + 1], 1.0)
            sb_v.append(vt)

        # ---- attention (per batch, both local heads packed) ----
        sb_att = {}
        for b in range(B):
            for ib in range(T // TI):
                av = [psV.tile([HD + 1, TI], f32, tag="av",
                               name=f"av_{b}_{ib}_{h}") for h in range(2)]
                njb = 4 * ib + 4
                qt = sb_qk[(0, (b * T + ib * TI) // TI)]
                for jb in range(njb):
                    gk = b * T + jb * TJ
                    kt = sb_qk[(1, gk // TI)]
                    ko = gk % TI
                    ss = []
                    for h in range(2):
                        s = psS.tile([128, TI], f32, tag="s")
                        nc.tensor.matmul(
                            s[:],
                            kt[h * HD:(h + 1) * HD, ko:ko + TJ],
                            qt[h * HD:(h + 1) * HD, :],
                            start=True, stop=True,
                            tile_position=(64 * h, 0))
                        ss.append(s)
                    vt = sb_v[gk // TJ]
                    for h in range(2):
                        p = pp.tile([TJ, TI], bf16, tag="p")
                        nc.scalar.activation(p[:], ss[h][:], Exp, scale=0.125)
                        if jb >= 4 * ib:
                            a = jb - 4 * ib
                            nc.vector.tensor_mul(
                                p[:], p[:], sb_mask[:, a * TI:(a + 1) * TI])
                        nc.tensor.matmul(
                            av[h][:], vt[:, h:h + 1, :], p[:],
                            start=(jb == 0), stop=(jb == njb - 1))
                att = pat.tile([128, TI], bf16, tag="att")
                for h in range(2):
                    rec = pr.tile([1, TI], f32, tag="rec")
                    nc.vector.reciprocal(rec[:], av[h][HD:HD + 1, :])
                    rb = pr.tile([HD, TI], f32, tag="rb")
                    nc.gpsimd.partition_broadcast(rb[:], rec[:], channels=HD)
                    nc.vector.tensor_mul(att[h * HD:(h + 1) * HD, :],
                                         av[h][0:HD, :], rb[:])
                sb_att[(b, ib)] = att

        # ---- output projection (partial y; host sums across cores) ----
        for tt in range(BT // TJ):
            g = tt * TJ
            at = sb_att[(g // T, (g % T) // TI)]
            ao = g % TI
            for et in range(2):
                po = psA.tile([TJ, TI], f32, tag="ps")
                nc.tensor.matmul(po[:], at[:, ao:ao + TJ],
                                 sb_wout[:, et * TI:(et + 1) * TI],
                                 start=True, stop=True)
                yo = pp.tile([TJ, TI], f32, tag="yo")
                nc.scalar.copy(yo[:], po[:])
                nc.sync.dma_start(
                    y_d.ap()[tt * TJ:(tt + 1) * TJ, et * TI:(et + 1) * TI],
                    yo[:])

    nc.compile()
    return nc


def _get_nc():
    if "nc" not in _built:
        _built["nc"] = _build()
    return _built["nc"]


def make_inputs(x, rotary_emb, Wqkv, Wout):
    """Host-side sharding/layout prep -> per-core in_maps."""
    x = np.asarray(x, dtype=np.float32)
    rotary_emb = np.asarray(rotary_emb, dtype=np.float32)
    Wqkv = np.asarray(Wqkv, dtype=np.float32)
    Wout = np.asarray(Wout, dtype=np.float32)

    xT = np.ascontiguousarray(x.reshape(BT, D).T).astype(BF16)

    cosT = np.ascontiguousarray(np.cos(rotary_emb).T)        # [64, 2048]
    ssT = np.ascontiguousarray(np.sin(rotary_emb).T)
    ssT[:HD // 2] *= -1.0                                    # signed sin
    cosb = np.tile(np.concatenate([cosT, cosT], 0), (1, B)).astype(np.float32)
    sinb = np.tile(np.concatenate([ssT, ssT], 0), (1, B)).astype(np.float32)

    jj = np.arange(TJ)[:, None]
    ii = np.arange(TI)[None, :]
    maskt = np.concatenate(
        [((a * TJ + jj) <= ii) for a in range(4)], axis=1).astype(BF16)

    in_maps = []
    for c in range(NCORES):
        r0 = c * DH
        wqkT = np.ascontiguousarray(
            np.concatenate([Wqkv[r0:r0 + DH], Wqkv[D + r0:D + r0 + DH]],
                           0).T).astype(BF16)
        wvT = np.ascontiguousarray(
            Wqkv[2 * D + r0:2 * D + r0 + DH].T).astype(BF16)
        woutT = np.ascontiguousarray(Wout[:, r0:r0 + DH].T).astype(BF16)
        in_maps.append({
            "xT": xT, "wqkT": wqkT, "wvT": wvT, "woutT": woutT,
            "cosb": cosb, "sinb": sinb, "maskt": maskt,
        })
    return in_maps


def kernel(x, rotary_emb, Wqkv, Wout):
    from concourse.bass_utils import run_bass_kernel_spmd

    nc = _get_nc()
    in_maps = make_inputs(x, rotary_emb, Wqkv, Wout)
    res = run_bass_kernel_spmd(nc, in_maps, core_ids=list(range(NCORES)))
    y = res.results[0]["y"].astype(np.float32, copy=True)
    for c in range(1, NCORES):
        y += res.results[c]["y"]
    return y.reshape(B, T, D)
